# revision 4
# baseline (speedup 1.0000x reference)
"""Linear-attention MultiHeadAttentionBlock kernel for 8 Trainium2 NeuronCores.

Sharding: core c handles (batch b = c//2, head-group g = c%2), 8 heads =
512 of 1024 d_model dims each; host sums the two per-batch partials.

fp8 strategy (error budget 2e-2; measured l2 ~9.7e-3 in CoreSim):
  Q proj, K proj, and the output GEMM Y run in float8_e4m3 DoubleRow
  mode (2 contraction rows per PE pass = 2x MAC rate, measured on HW).
  The V path (Vp projection, KV^T accumulation), Z normalization, and
  M fold stay bf16: V-path fp8 costs ~3% l2 (error flows straight
  through KV), while Q/K fp8 errors largely cancel through the Z
  normalization (~0.5% each) and the Y fp8 error is damped by the
  positive Z-weighted averaging (~0.8%).

fp8 DoubleRow operand layout: contraction pairs are adjacent blocks in
the free dim ([128, 2, N] APs); host packs x/w accordingly with static
scales (x*8, w*256; known randn input distributions).  The projection
PSUM descale (1/2048) folds into the feature-map ops; the qfz scale S1
folds into ksum2 and is divided out of y during the final PSUM copy.

Feature map: elu(x)+1 = min(exp(x),1) + max(x,0) — one ACT pass (the
only exp engine, 1.2GHz) + a max and a fused min-add on DVE/Pool.
Engine budget per phase is balanced so ACT (exps), DVE (0.96GHz, 2x on
16-bit), and Pool each stay under the phase's PE wall:
  Q: round A fmaps direct from PSUM; round B released (1 copy) and
     elu-finished during V nts 0-7.
  K: direct fmaps; max ops alternate DVE/Pool.
  V: vp copies round-robin ACT/DVE/Pool; Z-prep at nt8, zr+broadcast
     nt9-10, qfz = qft*zrp (PSUM read, fp8 write) at nts 11-14.
  Y: full-width [128,1024] PSUM->bf16 copies round-robin 3 engines.
"""

import numpy as np

import concourse.bass as bass
import concourse.mybir as mybir
import concourse.tile as tile
from concourse import bacc
from concourse.bass_utils import run_bass_kernel_spmd
from concourse.masks import make_identity

P = 128
L = 2048          # sequence length
DM = 1024         # d_model (= contraction dim of projections)
DG = 512          # per-core head-group width (8 heads x 64)
NT = L // P       # 16 n-tiles
KC = DM // P      # 8 contraction chunks
KP = KC // 2      # 4 contraction pair-chunks (fp8 DoubleRow)
DT = DG // P      # 4 d'-tiles (2 heads each)
NCH = 4           # n-chunks of 512 for transposed-Q projection
F32 = mybir.dt.float32
BF16 = mybir.dt.bfloat16
F8 = mybir.dt.float8e4
DR = mybir.MatmulPerfMode.DoubleRow

S1 = 16384.0      # qfz scale (Qf*zr ~ 1e-5..1e-4 -> fp8 normal range)
XS = 8.0          # x fp8 scale (values ~N(0,1))
WS = 256.0        # w fp8 scale (values ~N(0,1/1024))

_CACHE = {}


def build_nc(repeats=1, dma_once=False, phases='full'):
    nc = bacc.Bacc(None, target_bir_lowering=False)

    # fp8 x pair-chunks: [m][p][i][n] = xT[(2m+i)*128+p, n] * XS
    xq_d = nc.dram_tensor("xq8", [KP, P, 2, L], F8, kind="ExternalInput")
    xk_d = nc.dram_tensor("xk8", [KP, P, 2, L], F8, kind="ExternalInput")
    xv_d = nc.dram_tensor("xvT", [DM, L], BF16, kind="ExternalInput")
    # wq8: [p][2m + 8*(dt//2) + i][(dt%2)*128 + d']
    wq_d = nc.dram_tensor("wq8", [P, 2 * KC, DG // 2], F8, kind="ExternalInput")
    # wk8: [p][2m+i][d'] = wkT[(2m+i)*128+p, d'] * WS
    wk_d = nc.dram_tensor("wk8", [P, KC, DG], F8, kind="ExternalInput")
    wv_d = nc.dram_tensor("wvC", [P, KC * DG], BF16, kind="ExternalInput")
    wo_d = nc.dram_tensor("woC", [P, DT * DM], BF16, kind="ExternalInput")
    sel_d = nc.dram_tensor("sel8", [8, DT * P], BF16, kind="ExternalInput")
    y_d = nc.dram_tensor("y", [L, DM], BF16, kind="ExternalOutput")

    with tile.TileContext(nc) as tc:
        with (
            tc.tile_pool(name="const", bufs=1) as cpool,
            tc.tile_pool(name="x8", bufs=8) as x8p,      # fp8 [128,2,2048]
            tc.tile_pool(name="xv", bufs=8) as xvp,      # bf16 [128,2048]
            tc.tile_pool(name="w8", bufs=2) as w8p,      # fp8 weight cats
            tc.tile_pool(name="wt", bufs=2) as wtp,      # bf16 weight cats
            tc.tile_pool(name="qft", bufs=16) as qftp,   # QfT bf16
            tc.tile_pool(name="qfz", bufs=8) as qfzp,    # QfT*zr fp8 pairs
            tc.tile_pool(name="kf", bufs=16) as kfp,     # Kf bf16
            tc.tile_pool(name="vp", bufs=4) as vpp,      # Vp rotating
            tc.tile_pool(name="tmp", bufs=4) as tmp,     # feature-map temps
            tc.tile_pool(name="kvsb", bufs=1) as kvsb,   # ksum2/zr/kvc2
            tc.tile_pool(name="m8", bufs=2) as m8p,      # M fp8 pair tiles
            tc.tile_pool(name="ysb", bufs=4) as ysb,     # y row-tiles
            tc.tile_pool(name="pp", bufs=4, space="PSUM") as pp,
            tc.tile_pool(name="kvp", bufs=4, space="PSUM") as kvp,
        ):
            for _rep in range(repeats):
                body(nc, tc, cpool, x8p, xvp, w8p, wtp, qftp, qfzp, kfp,
                     vpp, tmp, kvsb, m8p, ysb, pp, kvp,
                     xq_d, xk_d, xv_d, wq_d, wk_d, wv_d, wo_d, sel_d, y_d,
                     first=(_rep == 0),
                     do_dma=(_rep == 0 or not dma_once), phases=phases)

    nc.compile()
    return nc


def body(nc, tc, cpool, x8p, xvp, w8p, wtp, qftp, qfzp, kfp,
         vpp, tmp, kvsb, m8p, ysb, pp, kvp,
         xq_d, xk_d, xv_d, wq_d, wk_d, wv_d, wo_d, sel_d, y_d, first=True,
         do_dma=True, phases='full'):
    Exp = mybir.ActivationFunctionType.Exp
    Alu = mybir.AluOpType

    # ---------------- Phase Q DMAs first (startup critical path) -------
    if do_dma:
        wq_t = w8p.tile([P, 2 * KC, DG // 2], F8, tag="w8", name="wq_t")
        nc.sync.dma_start(wq_t[:, 0:8, :], wq_d[:, 0:8, :])
        xq_c = []
        for m in range(KP):
            t = x8p.tile([P, 2, L], F8, tag="x8", name="xq8t")
            nc.sync.dma_start(t[:], xq_d[m, :, :, :])
            xq_c.append(t)
            if m == 0:
                nc.sync.dma_start(wq_t[:, 8:16, :], wq_d[:, 8:16, :])
        _CACHE["_qin"] = (wq_t, xq_c)
    else:
        wq_t, xq_c = _CACHE["_qin"]
    if first:
        ident_b = cpool.tile([P, P], BF16, name="ident_b")
        make_identity(nc, ident_b[:])
        ones = cpool.tile([P, 2], BF16, name="ones")
        ones_f = cpool.tile([P, 2], F32, name="ones_f")
        nc.gpsimd.memset(ones_f[:], 1.0)
        nc.vector.tensor_copy(ones[:], ones_f[:])
        _CACHE["_const"] = (ones, ident_b)
    else:
        ones, ident_b = _CACHE["_const"]

    if first:
        # p-state warmup while the first DMAs land: rotating transposes
        # keep the PE continuously busy (~3us) so it reaches full clock.
        warm = [kvp.tile([P, P], BF16, tag="acc", name=f"warm{_i}")
                for _i in range(4)]
        for i in range(24):
            nc.tensor.transpose(warm[i % 4][:], ident_b[:], ident_b[:])

    def wq_s(m, dt):
        # [128, 2, 128] stationary slice for pair m, d'-tile dt
        return wq_t[:, 2 * m + 8 * (dt // 2):2 * m + 8 * (dt // 2) + 2,
                    (dt % 2) * P:(dt % 2 + 1) * P]

    DS = 1.0 / (XS * WS)   # descale fp8-projection PSUM (x*XS @ w*WS)

    Relu = mybir.ActivationFunctionType.Relu

    def fmap_direct(ps, dst, act_max=False):
        # dst = elu(ps*DS)+1 = min(exp(ps*DS),1) + max(ps*DS,0)
        # GPSIMD can't touch PSUM and walrus only lowers the fused
        # min-add (scalar_tensor_tensor) on DVE, so: ACT exp (+ some
        # maxes as Relu to balance), DVE the rest.
        t1 = tmp.tile([P, 512], BF16, tag="t1", bufs=8, name="t1")
        t2 = tmp.tile([P, 512], BF16, tag="t2", bufs=8, name="t2")
        nc.scalar.activation(t1[:], ps[:], Exp, scale=DS)
        if act_max:
            nc.scalar.activation(t2[:], ps[:], Relu, scale=DS)
        else:
            nc.vector.tensor_scalar(t2[:], ps[:], DS, 0.0, Alu.mult, Alu.max)
        nc.vector.scalar_tensor_tensor(dst[:], t1[:], 1.0, t2[:],
                                       Alu.min, Alu.add)

    # Release-then-finish variant: one descaling copy frees the PSUM
    # bank now; the elu runs later on bf16 tiles (deferred to V phase).
    def fmap_release(ps, dst, use_act):
        if use_act:
            nc.scalar.mul(dst[:], ps[:], DS)
        else:
            nc.vector.tensor_scalar(dst[:], ps[:], DS, None, Alu.mult)

        def finish():
            t1 = tmp.tile([P, 512], BF16, tag="t1", bufs=8, name="t1")
            t2 = tmp.tile([P, 512], BF16, tag="t2", bufs=8, name="t2")
            nc.scalar.activation(t1[:], dst[:], Exp)
            nc.vector.tensor_scalar(t2[:], dst[:], 0.0, None, Alu.max)
            nc.vector.scalar_tensor_tensor(dst[:], t1[:], 1.0, t2[:],
                                           Alu.min, Alu.add)
        return finish

    # ---------------- Phase Q: QfT (transposed, fp8 DoubleRow) ---------
    # Round A covers n 0:1024 kc-pair-outer across 8 PSUM banks (each
    # arriving chunk fully consumed); its fmaps run direct from PSUM.
    # Round B (all chunks resident) covers n 1024:2048 dt-outer two
    # banks at a time; released by one copy, elu-finished in V.
    qft = [None] * 16  # bf16 [128, 512]: index dt*4 + nch
    psb = []
    for dt in range(DT):
        pool, tag = (pp, "pp") if dt < 2 else (kvp, "acc")
        psb.append([pool.tile([P, 512], F32, tag=tag, name=f"q{dt}")
                    for _w in range(2)])
    for m in range(KP):
        for dt in range(DT):
            for win in range(2):
                for h in range(2):
                    # one accumulation group per PSUM bank (2KB zero
                    # region): only the first matmul starts, last stops
                    nc.tensor.matmul(
                        psb[dt][win][:, h * 256:(h + 1) * 256],
                        wq_s(m, dt),
                        xq_c[m][:, :, win * 512 + h * 256:
                                win * 512 + (h + 1) * 256],
                        start=(m == 0 and h == 0),
                        stop=(m == KP - 1 and h == 1),
                        perf_mode=DR,
                    )
    for dt in range(DT):
        for win in range(2):
            qf = qftp.tile([P, 512], BF16, tag="qft")
            fmap_direct(psb[dt][win], qf)
            qft[dt * NCH + win] = qf
    finishers = []
    for dt in range(DT):
        ps0 = pp.tile([P, 512], F32, tag="pp", name="ps0")
        ps1 = pp.tile([P, 512], F32, tag="pp", name="ps1")
        for m in range(KP):
            for h in range(2):
                nc.tensor.matmul(
                    ps0[:, h * 256:(h + 1) * 256], wq_s(m, dt),
                    xq_c[m][:, :, 1024 + h * 256:1024 + (h + 1) * 256],
                    start=(m == 0 and h == 0),
                    stop=(m == KP - 1 and h == 1), perf_mode=DR,
                )
                nc.tensor.matmul(
                    ps1[:, h * 256:(h + 1) * 256], wq_s(m, dt),
                    xq_c[m][:, :, 1536 + h * 256:1536 + (h + 1) * 256],
                    start=(m == 0 and h == 0),
                    stop=(m == KP - 1 and h == 1), perf_mode=DR,
                )
        for win, ps in ((0, ps0), (1, ps1)):
            qf = qftp.tile([P, 512], BF16, tag="qft")
            finishers.append(fmap_release(ps, qf, use_act=(win == 0)))
            qft[dt * NCH + 2 + win] = qf

    def qft_block(dt, nt):
        t = qft[dt * NCH + (nt * P) // 512]
        off = (nt * P) % 512
        return t[:, off:off + P]

    if phases == 'q':
        yt = ysb.tile([P, DM], BF16, tag="ysb", name="yt")
        nc.vector.tensor_copy(yt[:, 0:512], qft[15][:])
        nc.sync.dma_start(y_d[0:P, :], yt[:])
        return

    # ---------------- Phase K: Kf (n-space, fp8 DoubleRow) -------------
    if do_dma:
        wk_t = w8p.tile([P, KC, DG], F8, tag="w8", name="wk_t")
        nc.sync.dma_start(wk_t[:], wk_d[:, :, :])
        xk_c = []
        for m in range(KP):
            t = x8p.tile([P, 2, L], F8, tag="x8", name="xk8t")
            nc.sync.dma_start(t[:], xk_d[m, :, :, :])
            xk_c.append(t)
        sel_t = cpool.tile([8, DT * P], BF16, name="sel_t")
        nc.sync.dma_start(sel_t[:], sel_d[:, :])
        _CACHE["_kin"] = (wk_t, xk_c, sel_t)
    else:
        wk_t, xk_c, sel_t = _CACHE["_kin"]

    kf = []
    ksumb = [kvp.tile([P, 2], F32, tag="acc", name=f"ksumb{_d}")
             for _d in range(DT)]

    def ksum_mm(nt):
        for dt in range(DT):
            nc.tensor.matmul(
                ksumb[dt][:],
                kf[nt][:, dt * P:(dt + 1) * P],
                ones[:],
                start=(nt == 0), stop=(nt == NT - 1),
            )

    for nt in range(NT):
        ps = pp.tile([P, 512], F32, tag="pp")
        for m in range(KP):
            for win in range(2):
                nc.tensor.matmul(
                    ps[:, win * 256:(win + 1) * 256],
                    xk_c[m][:, :, nt * P:(nt + 1) * P],
                    wk_t[:, 2 * m:2 * m + 2, win * 256:(win + 1) * 256],
                    start=(m == 0 and win == 0),
                    stop=(m == KP - 1 and win == 1),
                    perf_mode=DR,
                )
        kft = kfp.tile([P, 512], BF16, tag="kf")
        fmap_direct(ps, kft, act_max=(nt % 4 == 3))
        kf.append(kft)
        if nt > 3:
            ksum_mm(nt - 4)
    for nt_ in (NT - 4, NT - 3, NT - 2, NT - 1):
        ksum_mm(nt_)

    # ksum2 columns from ksumb, scaled by 1/S1 so zpre lands at S1/zr
    # (frees the kvp ring for kvt accumulators)
    ksum2 = kvsb.tile([P, 2 * DT], BF16, tag="ksum2")
    for dt in range(DT):
        nc.scalar.mul(ksum2[0:64, 2 * dt:2 * dt + 1],
                      ksumb[dt][0:64, 0:1], 1.0 / S1)
        nc.scalar.mul(ksum2[64:128, 2 * dt:2 * dt + 1],
                      ksumb[dt][64:128, 0:1], 0.0)
        nc.scalar.mul(ksum2[0:64, 2 * dt + 1:2 * dt + 2],
                      ksumb[dt][0:64, 0:1], 0.0)
        nc.scalar.mul(ksum2[64:128, 2 * dt + 1:2 * dt + 2],
                      ksumb[dt][64:128, 0:1], 1.0 / S1)

    if phases == 'qk':
        yt = ysb.tile([P, DM], BF16, tag="ysb", name="yt")
        nc.vector.tensor_copy(yt[:, 0:512], kf[15][:])
        nc.sync.dma_start(y_d[0:P, :], yt[:])
        return

    # ---------------- Phase V + KV^T accumulation (bf16) ---------------
    if do_dma:
        wv_t = wtp.tile([P, KC * DG], BF16, tag="wt", name="wv_t")
        nc.sync.dma_start(wv_t[:], wv_d[:, :])
        xv_c = []
        for kc in range(KC):
            t = xvp.tile([P, L], BF16, tag="xv", name="xvt")
            nc.sync.dma_start(t[:], xv_d[kc * P:(kc + 1) * P, :])
            xv_c.append(t)
        wo_t = wtp.tile([P, DT * DM], BF16, tag="wt", name="wo_t")
        nc.sync.dma_start(wo_t[:], wo_d[:, :])
        _CACHE["_vin"] = (wv_t, xv_c, wo_t)
    else:
        wv_t, xv_c, wo_t = _CACHE["_vin"]

    kvc2 = []
    for c in range(DT):
        kvc = kvsb.tile([P, P], BF16, tag="kvcat", bufs=4)
        nc.gpsimd.memset(kvc[0:64, 64:128], 0.0)
        nc.gpsimd.memset(kvc[64:128, 0:64], 0.0)
        kvc2.append(kvc)

    kvt = [kvp.tile([P, P], F32, tag="acc", name=f"kvt{_c}")
           for _c in range(DT)]
    vps = []

    def kvt_mm(nt):
        for c in range(DT):
            nc.tensor.matmul(
                kvt[c][:],
                vps[nt][:, c * P:(c + 1) * P],
                kf[nt][:, c * P:(c + 1) * P],
                start=(nt == 0), stop=(nt == NT - 1),
            )

    # qfz fp8 pair tiles: [pair t][nch] halves c=2t+i along dim 1
    qfz = [[None] * NCH for _ in range(2)]
    for t_ in range(2):
        for nch in range(NCH):
            qfz[t_][nch] = qfzp.tile([P, 2, 512], F8, tag="qfz",
                                     name=f"qfz{t_}_{nch}")

    zbank = None
    zrb = None
    zrA = None
    for nt in range(NT):
        ps = pp.tile([P, 512], F32, tag="pp")
        for kc in range(KC):
            nc.tensor.matmul(
                ps[:],
                xv_c[kc][:, nt * P:(nt + 1) * P],
                wv_t[:, kc * DG:(kc + 1) * DG],
                start=(kc == 0), stop=(kc == KC - 1),
            )
        vp_t = vpp.tile([P, 512], BF16, tag="vp")
        # split halves across ACT+DVE: halves the copy latency so the
        # kvt stationary is ready well before the PE needs it
        nc.scalar.copy(vp_t[:, 0:256], ps[:, 0:256])
        nc.vector.tensor_copy(vp_t[:, 256:512], ps[:, 256:512])
        vps.append(vp_t)
        # two tiles of slack: the PE never waits on the vp copy
        if nt > 1 and 'nokvt' not in phases:
            kvt_mm(nt - 2)
        if nt < len(finishers) and 'nofin' not in phases:
            # deferred round-B Q elu (one per early-V tile; all done
            # before the Z-prep at nt 8)
            finishers[nt]()
        if 'noz' in phases or 'nofin' in phases:
            pass
        elif nt == 9:
            zbank = pp.tile([P, P], F32, tag="pp", name="zbank")
            idx = 0
            for nt_ in range(NT):
                for dt in range(DT):
                    ccol = nt_ * 8 + dt * 2
                    nc.tensor.matmul(
                        zbank[:, ccol:ccol + 2],
                        qft_block(dt, nt_),
                        ksum2[:, 2 * dt:2 * dt + 2],
                        start=(idx == 0), stop=(idx == NT * DT - 1),
                        skip_group_check=True,
                    )
                    idx += 1
        elif nt == 10:
            zrb = kvsb.tile([P, P], BF16, tag="zrb", name="zrb")
            with nc.allow_low_precision(reason="zr broadcast is bf16 anyway"):
                nc.vector.reciprocal(zrb[:], zbank[:])
            zrA = kvsb.tile([8, L], BF16, tag="zrA", name="zrA")
        elif nt in (11, 12):
            # transpose zrb columns into one [8, 1024] PSUM tile (8
            # transposes, one zero-region group) then one DVE copy
            g = nt - 11
            ztp = pp.tile([8, 2 * DG], BF16, tag="pp", name=f"ztp{g}")
            for j in range(2):
                nch = g * 2 + j
                for q in range(4):
                    nt_ = nch * 4 + q
                    nc.tensor.matmul(
                        ztp[:, (j * 4 + q) * P:(j * 4 + q + 1) * P],
                        zrb[:, nt_ * 8:(nt_ + 1) * 8], ident_b[:],
                        is_transpose=True, skip_group_check=True,
                        start=(j == 0 and q == 0), stop=(j == 1 and q == 3),
                    )
            nc.vector.tensor_copy(zrA[:, g * 1024:(g + 1) * 1024], ztp[:])
        elif nt in (13, 14, 15):
            nch = nt - 13
            # two zrp tiles alternate within the round: fewer PSUM ring
            # allocations, and the PE pipelines past the DVE multiplies
            zrps = [pp.tile([P, 512], F32, tag="pp", name=f"zrp{_j}")
                    for _j in range(2)]
            for dt in range(DT):
                zrp = zrps[dt % 2]
                nc.tensor.matmul(
                    zrp[:], sel_t[:, dt * P:(dt + 1) * P],
                    zrA[:, nch * 512:(nch + 1) * 512],
                    start=True, stop=True,
                )
                # qfz half (pair t = dt//2, block i = dt%2) <- qft * zr.
                # DVE is the V-phase's busiest engine, so half the
                # multiplies route ACT (PSUM->SBUF copy) + Pool (SBUF
                # multiply) instead -- GPSIMD cannot read PSUM directly.
                dst = qfz[dt // 2][nch][:, dt % 2, :]
                if dt % 2 == 0:
                    nc.vector.tensor_tensor(
                        dst, qft[dt * NCH + nch][:], zrp[:], Alu.mult)
                else:
                    zrs = tmp.tile([P, 512], BF16, tag="zrs", bufs=4,
                                   name="zrs")
                    nc.scalar.copy(zrs[:], zrp[:])
                    nc.gpsimd.tensor_tensor(
                        dst, qft[dt * NCH + nch][:], zrs[:], Alu.mult)

    if 'nokvt' not in phases:
        kvt_mm(NT - 2)
        kvt_mm(NT - 1)

    if phases.startswith('qkv'):
        yt = ysb.tile([P, DM], BF16, tag="ysb", name="yt")
        nc.vector.tensor_copy(yt[:, 0:512], vps[15][:])
        nc.sync.dma_start(y_d[0:P, :], yt[:])
        return

    # last qfz round (nch=3): overlaps the M phase; Y reads it only
    # from its 13th tile onward
    for dt in range(DT):
        zrpL = pp.tile([P, 512], F32, tag="pp", name="zrpL")
        nc.tensor.matmul(
            zrpL[:], sel_t[:, dt * P:(dt + 1) * P],
            zrA[:, 3 * 512:4 * 512],
            start=True, stop=True,
        )
        if dt % 2 == 0:
            nc.vector.tensor_tensor(qfz[dt // 2][3][:, dt % 2, :],
                                    qft[dt * NCH + 3][:], zrpL[:], Alu.mult)
        else:
            zrsL = tmp.tile([P, 512], BF16, tag="zrs", bufs=4, name="zrsL")
            nc.scalar.copy(zrsL[:], zrpL[:])
            nc.gpsimd.tensor_tensor(qfz[dt // 2][3][:, dt % 2, :],
                                    qft[dt * NCH + 3][:], zrsL[:], Alu.mult)

    # ---------------- M = blockdiag(KV^T)^T @ Wo rows (bf16 -> fp8) ----
    # m8[t] [128, 2, 1024]: half i holds M rows for chunk c = 2t+i.
    m8 = [m8p.tile([P, 2, DM], F8, tag="m8", name=f"m8_{_t}")
          for _t in range(2)]
    for c in range(DT):
        nc.scalar.copy(kvc2[c][0:64, 0:64], kvt[c][0:64, 0:64])
        nc.vector.tensor_copy(kvc2[c][64:128, 64:128], kvt[c][64:128, 64:128])
        for hhalf in range(2):
            mps = pp.tile([P, 512], F32, tag="pp", name="mps")
            nc.tensor.matmul(
                mps[:], kvc2[c][:],
                wo_t[:, c * DM + hhalf * 512:c * DM + (hhalf + 1) * 512],
                start=True, stop=True,
            )
            dst = m8[c // 2][:, c % 2, hhalf * 512:(hhalf + 1) * 512]
            if hhalf == 0:
                nc.vector.tensor_copy(dst, mps[:])
            else:
                nc.scalar.copy(dst, mps[:])

    # ---------------- Phase Y: y = (qfz^T @ M8) / S1 (fp8 DoubleRow) ---
    for nt in range(NT):
        nch = (nt * P) // 512
        off = (nt * P) % 512
        ypool, ytag = ((kvp, "acc") if nt % 2 == 0 else (pp, "pp"))
        yps0 = ypool.tile([P, 512], F32, tag=ytag, name="yps0")
        yps1 = ypool.tile([P, 512], F32, tag=ytag, name="yps1")
        yt = ysb.tile([P, DM], BF16, tag="ysb", name="yt")
        # t_-outer: each qfz stationary load serves 4 matmuls (both
        # PSUM banks' windows)
        for t_ in range(2):
            for win in range(4):
                dst = yps0 if win < 2 else yps1
                nc.tensor.matmul(
                    dst[:, (win % 2) * 256:(win % 2 + 1) * 256],
                    qfz[t_][nch][:, :, off:off + P],
                    m8[t_][:, :, win * 256:(win + 1) * 256],
                    start=(t_ == 0 and win % 2 == 0),
                    stop=(t_ == 1 and win % 2 == 1), perf_mode=DR,
                )
        # descale copies: one half per engine each nt (ACT + DVE run
        # concurrently; a single engine per nt would pace Y at ~1.3us/nt)
        nc.scalar.mul(yt[:, 0:512], yps0[:], 1.0 / S1)
        nc.vector.tensor_scalar(yt[:, 512:1024], yps1[:], 1.0 / S1,
                                None, Alu.mult)
        if nt == NT - 1:
            nc.sync.dma_start(y_d[nt * P:(nt + 1) * P, 0:512],
                              yt[:, 0:512])
            nc.sync.dma_start(y_d[nt * P:(nt + 1) * P, 512:1024],
                              yt[:, 512:1024])
        else:
            nc.sync.dma_start(y_d[nt * P:(nt + 1) * P, :], yt[:])


def make_in_maps(q, k, v, w_q, w_k, w_v, w_o):
    import ml_dtypes
    F8NP = ml_dtypes.float8_e4m3
    q = np.asarray(q, dtype=np.float32)
    k = np.asarray(k, dtype=np.float32)
    v = np.asarray(v, dtype=np.float32)
    w_q = np.asarray(w_q, dtype=np.float32)
    w_k = np.asarray(w_k, dtype=np.float32)
    w_v = np.asarray(w_v, dtype=np.float32)
    w_o = np.asarray(w_o, dtype=np.float32)
    B = q.shape[0]

    def x8pack(x):
        # [L, DM] -> fp8 [KP, P, 2, L]: [m][p][i][n] = x.T[(2m+i)*128+p, n]*XS
        xT = (x.T * XS).astype(F8NP)                 # [DM, L]
        return np.ascontiguousarray(
            xT.reshape(KP, 2, P, L).swapaxes(1, 2))  # [KP, P, 2, L]

    def wq8pack(w, g):
        # [128, 16, 128]: [p][2m + 8*(dt//2) + i][(dt%2)*128 + d']
        wT = (w[g * DG:(g + 1) * DG, :].T * WS).astype(F8NP)  # [DM, DG]
        out = np.zeros((P, 2 * KC, DG // 2), dtype=F8NP)
        for m in range(KP):
            for i in range(2):
                for dt in range(DT):
                    blk = wT[(2 * m + i) * P:(2 * m + i + 1) * P,
                             dt * P:(dt + 1) * P]
                    out[:, 2 * m + 8 * (dt // 2) + i,
                        (dt % 2) * P:(dt % 2 + 1) * P] = blk
        return out

    def wk8pack(w, g):
        # [128, 8, 512]: [p][2m+i][d'] = wkT[(2m+i)*128+p, d']
        wT = (w[g * DG:(g + 1) * DG, :].T * WS).astype(F8NP)  # [DM, DG]
        return np.ascontiguousarray(wT.reshape(KC, P, DG).swapaxes(0, 1))

    def wcat16(w, g):
        wT = w[g * DG:(g + 1) * DG, :].T
        return np.concatenate(
            [wT[kc * P:(kc + 1) * P, :] for kc in range(KC)],
            axis=1).astype(ml_dtypes.bfloat16)

    def wocat(w, g):
        woT = w[:, g * DG:(g + 1) * DG].T
        return np.concatenate(
            [woT[c * P:(c + 1) * P, :] for c in range(DT)],
            axis=1).astype(ml_dtypes.bfloat16)

    xq8 = [x8pack(q[b]) for b in range(B)]
    xk8 = [x8pack(k[b]) for b in range(B)]
    xvT = [np.ascontiguousarray(v[b].T).astype(ml_dtypes.bfloat16)
           for b in range(B)]
    wq8 = [wq8pack(w_q, g) for g in range(2)]
    wk8 = [wk8pack(w_k, g) for g in range(2)]
    wv16 = [wcat16(w_v, g) for g in range(2)]
    wo16 = [wocat(w_o, g) for g in range(2)]
    sel8 = np.zeros((8, DT * P), dtype=np.float32)
    for dt in range(4):
        sel8[2 * dt, dt * P:dt * P + 64] = 1.0
        sel8[2 * dt + 1, dt * P + 64:(dt + 1) * P] = 1.0
    sel8 = sel8.astype(ml_dtypes.bfloat16)
    in_maps = []
    for c in range(8):
        b, g = c // 2, c % 2
        in_maps.append({
            "xq8": xq8[b], "xk8": xk8[b], "xvT": xvT[b],
            "wq8": wq8[g], "wk8": wk8[g], "wvC": wv16[g], "woC": wo16[g],
            "sel8": sel8,
        })
    return in_maps


def kernel(q, k, v, mask, w_q, w_k, w_v, w_o):
    if "nc" not in _CACHE:
        _CACHE["nc"] = build_nc()
    nc = _CACHE["nc"]
    in_maps = make_in_maps(q, k, v, w_q, w_k, w_v, w_o)
    res = run_bass_kernel_spmd(nc, in_maps, list(range(8)))
    _CACHE["last_results"] = res
    B = np.asarray(q).shape[0]
    out = np.empty((B, L, DM), dtype=np.float32)
    for b in range(B):
        out[b] = (res.results[2 * b]["y"].astype(np.float32)
                  + res.results[2 * b + 1]["y"].astype(np.float32))
    return out
